# revision 67
# baseline (speedup 1.0000x reference)
"""MLA (multi-head latent attention) Trainium2 kernel.

Sharding: 8 cores = 2 (batch) x 4 (head groups of 4 heads).
Each core computes, for its batch b and heads [4g, 4g+4):
  latents kv_d/q_d (replicated within the batch group), per-head
  up-projections + RoPE, causal SDPA, and a partial o_proj
  out_core[o, q] = sum_{d in core's 512 head-dims} W_o[o, d] * y[d, q].
Host sums the 4 partials per batch (the all-reduce step of the hint,
performed at unshard time) and transposes to [S, H].

All matmuls run in bf16 with fp32 PSUM accumulation.

Phase 1 is fused: one pass over xT (stored k-chunk/quarter-major as
[128, 64, 512]) computes rope-k and both latents per S-quarter, using
6 PSUM banks, so xT is read once and the PE never waits on a second
sweep. PSUM drains alternate Act/DVE so neither engine serializes the
pipeline. SDPA runs per 128-key chunk with deep PSUM buffering.
"""

import numpy as np
import ml_dtypes

import concourse.bass as bass
import concourse.mybir as mybir
import concourse.tile as tile
from concourse import bacc
from concourse._compat import get_trn_type
from concourse.bass_utils import run_bass_kernel_spmd

H = 2048
NH = 16
HD = 128           # head dim
RD = 64            # rotary dim
RH = 32            # rotary half
LAT = 256
B = 2
S = 2048
BASE = 10000.0
N_CORES = 8
HEADS_PER_CORE = 4
P = 128
NQB = S // 512     # 4 query blocks of 512
NKI = S // 128     # 16 key chunks of 128
SCALE = 1.0 / np.sqrt(float(HD))
EXP_BIAS = -4.0

BF16 = mybir.dt.bfloat16
F32 = mybir.dt.float32
_bf = ml_dtypes.bfloat16


def _mm(nc, out, lhsT, rhs, start, stop):
    nc.tensor.matmul(out, lhsT, rhs, start=start, stop=stop)


def _rope_k_pair(nc, rot, kT, kraw, cos_sb, sin_sb, nq, p, eng):
    """Apply rope to quarter nq of kraw for pair p on engine `eng`.
    Swaps go via SBUF-SBUF DMA on the sync queue."""
    qs = slice(nq * 512, (nq + 1) * 512)
    h0, h1 = 2 * p, 2 * p + 1
    swq = rot.tile([P, 512], BF16, tag="swq", name="swq", bufs=3)
    nc.sync.dma_start(swq[0:32, :], kraw[p][32:64, qs])
    nc.sync.dma_start(swq[32:64, :], kraw[p][0:32, qs])
    nc.sync.dma_start(swq[64:96, :], kraw[p][96:128, qs])
    nc.sync.dma_start(swq[96:128, :], kraw[p][64:96, qs])
    eng.tensor_mul(swq[:], swq[:], sin_sb[:, qs])
    eng.tensor_mul(kT[h1][0:64, qs], kraw[p][0:64, qs], cos_sb[0:64, qs])
    eng.tensor_add(kT[h1][0:64, qs], kT[h1][0:64, qs], swq[0:64, :])
    eng.tensor_mul(kT[h0][64:128, qs], kraw[p][64:128, qs],
                   cos_sb[64:128, qs])
    eng.tensor_add(kT[h0][64:128, qs], kT[h0][64:128, qs],
                   swq[64:128, :])


def _rope_k_quarter(nc, rot, kT, kraw, cos_sb, sin_sb, nq, eng):
    for p in range(2):
        _rope_k_pair(nc, rot, kT, kraw, cos_sb, sin_sb, nq, p, eng)


def build_program(nrep=1, bench_io=False):
    nc = bacc.Bacc(
        get_trn_type() or "TRN2",
        target_bir_lowering=False,
        debug=False,
        num_devices=N_CORES,
    )

    if bench_io:
        dummy = nc.declare_dram_parameter("bdummy", [1, 128], F32, isOutput=False)
        xTq = nc.dram_tensor("xTq", [P, 64, 512], BF16)
        w_kvd = nc.dram_tensor("w_kvd", [P, 16, LAT], BF16)
        w_qd = nc.dram_tensor("w_qd", [P, 16, LAT], BF16)
        w_rk = nc.dram_tensor("w_rk", [P, 16, 256], BF16)
        w_qc = nc.dram_tensor("w_qc", [P, 2, 512], BF16)
        w_kc = nc.dram_tensor("w_kc", [P, 2, 256], BF16)
        w_v = nc.dram_tensor("w_v", [P, 2, 512], BF16)
        w_o = nc.dram_tensor("w_o", [P, 4, H], BF16)
        cosA = nc.dram_tensor("cosA", [P, S], BF16)
        sinB = nc.dram_tensor("sinB", [P, S], BF16)
        masks = nc.dram_tensor("masks", [P, 4, 512], BF16)
        out = nc.dram_tensor("outs", [32, P, 1024], BF16)
        outp = nc.declare_dram_parameter("out", [1, 128], BF16, isOutput=True)
    else:
        xTq = nc.declare_dram_parameter("xTq", [P, 64, 512], BF16, isOutput=False)
        w_kvd = nc.declare_dram_parameter("w_kvd", [P, 16, LAT], BF16, isOutput=False)
        w_qd = nc.declare_dram_parameter("w_qd", [P, 16, LAT], BF16, isOutput=False)
        w_rk = nc.declare_dram_parameter("w_rk", [P, 16, 256], BF16, isOutput=False)
        w_qc = nc.declare_dram_parameter("w_qc", [P, 2, 512], BF16, isOutput=False)
        w_kc = nc.declare_dram_parameter("w_kc", [P, 2, 256], BF16, isOutput=False)
        w_v = nc.declare_dram_parameter("w_v", [P, 2, 512], BF16, isOutput=False)
        w_o = nc.declare_dram_parameter("w_o", [P, 4, H], BF16, isOutput=False)
        cosA = nc.declare_dram_parameter("cosA", [P, S], BF16, isOutput=False)
        sinB = nc.declare_dram_parameter("sinB", [P, S], BF16, isOutput=False)
        masks = nc.declare_dram_parameter("masks", [P, 4, 512], BF16, isOutput=False)
        out = nc.declare_dram_parameter("out", [32, P, 1024], BF16, isOutput=True)

    Exp = mybir.ActivationFunctionType.Exp

    scratch = (nc.dram_tensor("scratch", [32, P, 1024], BF16) if nrep > 1 else None)

    with tile.TileContext(nc) as tc:
      for rep in range(nrep):
        out_r = out if rep == nrep - 1 else scratch
        with (
            tc.tile_pool(name=f"wpool{rep}", bufs=1) as wpool,
            tc.tile_pool(name=f"main1_{rep}", bufs=1) as main1,
            tc.tile_pool(name=f"main2_{rep}", bufs=1) as main2,
            tc.tile_pool(name=f"ph1_{rep}", bufs=1) as ph1,
            tc.tile_pool(name=f"rot{rep}", bufs=3) as rot,
        ):
            # -------- persistent small tensors (DMAs deferred) --------
            cos_sb = wpool.tile([P, S], BF16, tag="cos", name="cos")
            sin_sb = wpool.tile([P, S], BF16, tag="sin", name="sin")
            mask_sb = wpool.tile([P, 4, 512], BF16, tag="mask", name="mask")
            ones_sb = wpool.tile([P, P], BF16, tag="ones", name="ones")
            nc.gpsimd.memset(ones_sb[:], 1.0)
            ebias_sb = wpool.tile([P, 1], F32, tag="ebias", name="ebias")
            nc.gpsimd.memset(ebias_sb[:], EXP_BIAS)
            wo_sb = wpool.tile([P, 4, H], BF16, tag="wo", name="wo")
            wqc_sb = wpool.tile([P, 2, 512], BF16, tag="wqc", name="wqc")
            wkc_sb = wpool.tile([P, 2, 256], BF16, tag="wkc", name="wkc")
            wv_sb = wpool.tile([P, 2, 512], BF16, tag="wv", name="wv")

            # -------- phase-1 outputs (latents + raw rope-k), bf16 --------
            kvd_sb = [main1.tile([P, S], BF16, tag=f"kvd{m}", name=f"kvd{m}")
                      for m in range(2)]
            qd_sb = [main1.tile([P, S], BF16, tag=f"qd{m}", name=f"qd{m}")
                     for m in range(2)]
            kraw = [main1.tile([P, S], BF16, tag=f"kraw{p}", name=f"kraw{p}")
                    for p in range(2)]

            # phase-1 weights, loaded JIT in k-groups during quarter 0
            wkvd_sb = ph1.tile([P, 16, LAT], BF16, tag="wkvd", name="wkvd")
            wqd_sb = ph1.tile([P, 16, LAT], BF16, tag="wqd", name="wqd")
            wrk_sb = ph1.tile([P, 16, 256], BF16, tag="wrk", name="wrk")

            # -------- per-head q/k tiles (dims on partitions), v, y --------
            # even head h: rows [0:64] content, [64:128] rope
            # odd  head h: rows [0:64] rope,    [64:128] content
            qT = [main2.tile([P, S], BF16, tag=f"qT{h}", name=f"qT{h}")
                  for h in range(4)]
            kT = [main2.tile([P, S], BF16, tag=f"kT{h}", name=f"kT{h}")
                  for h in range(4)]
            v_sb = [main2.tile([P, NKI, 256], BF16, tag=f"v{p}", name=f"v{p}")
                    for p in range(2)]
            y_sb = [main2.tile([P, S], BF16, tag=f"y{h}", name=f"y{h}")
                    for h in range(4)]

            # -------- phase 1 (fused): per S-quarter, one xT sweep feeds
            # rope-k (2 pair-groups) + latents (kvd x2, qd x2): 6 banks.
            # Phase 2 (k/q/v up-projections + both ropes) is folded in:
            # quarter nq's pair work is emitted one quarter later (kg==1
            # boundary), riding the PE-bound stream's Act/DVE slack, so
            # the old elementwise-bound pair phases disappear. ----
            wgl = [(wkvd_sb, 0), (wkvd_sb, 1), (wqd_sb, 0), (wqd_sb, 1)]
            targets = [kvd_sb[0], kvd_sb[1], qd_sb[0], qd_sb[1]]

            # -------- HAM warm-up --------
            # The PE sits idle through the ~6.5us framework preamble and
            # first-weight DMA wait, so the HAM clock-gate holds 1.2GHz
            # for the first ~5us of real matmuls. A dependency-free dummy
            # matmul stream (ones x whatever SBUF holds; result never
            # read) fills that dead window and trips the 4096-cycle
            # busy-window to 2.4GHz before the real stream starts.
            with tc.tile_pool(name=f"warm{rep}", bufs=1,
                              space="PSUM") as wps:
                wt = wps.tile([P, 512], F32, tag="warm", name="warm")
                for _ in range(9):
                    nc.tensor.matmul(wt[:], ones_sb[:], wo_sb[:, 0, 0:512],
                                     start=True, stop=True,
                                     skip_group_check=True)

            def q_rope_quarter(h, nq):
                # rope on q for head h, quarter nq (DVE; swaps via
                # SBUF-SBUF DMA on the scalar queue)
                r = 64 if h % 2 == 0 else 0
                qs = slice(nq * 512, (nq + 1) * 512)
                swp = rot.tile([P, 512], BF16, tag="swp", name="swp", bufs=3)
                nc.scalar.dma_start(swp[r:r + 32, :], qT[h][r + 32:r + 64, qs])
                nc.scalar.dma_start(swp[r + 32:r + 64, :], qT[h][r:r + 32, qs])
                nc.vector.tensor_mul(
                    qT[h][r:r + 64, qs], qT[h][r:r + 64, qs],
                    cos_sb[r:r + 64, qs]
                )
                nc.vector.tensor_mul(
                    swp[r:r + 64, :], swp[r:r + 64, :], sin_sb[r:r + 64, qs]
                )
                nc.vector.tensor_add(
                    qT[h][r:r + 64, qs], qT[h][r:r + 64, qs], swp[r:r + 64, :]
                )

            def pair_tiles(ps1, nq, tgs=("px", "py"), deferred=None,
                           tile_bufs=1):
                # Phase-2 work for column-quarter nq as 10 independent
                # thunks (2 k-content, 4 q+rope, 4 v), each one psum tile
                # on the ps1 pool's two spare banks (tags px/py). The
                # caller interleaves them between ph1 k-chunks so drains
                # always finish in the gaps.
                ns = slice(nq * 512, (nq + 1) * 512)
                thunks = []

                def kc_thunk(p, tag):
                    def run():
                        h0, h1 = 2 * p, 2 * p + 1
                        pt = ps1.tile([P, 512], F32, tag=tag,
                                      name=f"kc{p}_{nq}", bufs=tile_bufs)
                        for kc in range(2):
                            _mm(nc, pt[:],
                                wkc_sb[:, kc, p * 128:(p + 1) * 128],
                                kvd_sb[kc][:, ns], kc == 0, kc == 1)
                        if p == 0:
                            nc.scalar.copy(kT[h0][0:64, ns], pt[0:64, :])
                            nc.vector.tensor_copy(kT[h1][64:128, ns],
                                                  pt[64:128, :])
                        else:
                            nc.vector.tensor_copy(kT[h0][0:64, ns],
                                                  pt[0:64, :])
                            nc.scalar.copy(kT[h1][64:128, ns],
                                           pt[64:128, :])
                    return run

                def qt_thunk(h, tag):
                    def run():
                        qt = ps1.tile([P, 512], F32, tag=tag,
                                      name=f"qt{h}_{nq}", bufs=tile_bufs)
                        for kc in range(2):
                            _mm(nc, qt[:],
                                wqc_sb[:, kc, h * 128:(h + 1) * 128],
                                qd_sb[kc][:, ns], kc == 0, kc == 1)
                        if h % 2 == 0:
                            nc.scalar.copy(qT[h][:, ns], qt[:])
                        else:
                            nc.vector.tensor_copy(qT[h][:, ns], qt[:])
                        if deferred is None:
                            q_rope_quarter(h, nq)
                        else:
                            deferred.append(
                                lambda h=h: q_rope_quarter(h, nq))
                    return run

                def vt_thunk(p, half, tag):
                    def run():
                        s0 = 4 * nq + 2 * half
                        vt = ps1.tile([P, 512], F32, tag=tag,
                                      name=f"vt{p}_{nq}_{half}", bufs=tile_bufs)
                        for c2 in range(2):
                            for kc in range(2):
                                _mm(nc, vt[:, c2 * 256:(c2 + 1) * 256],
                                    kvd_sb[kc][:, (s0 + c2) * 128:
                                               (s0 + c2 + 1) * 128],
                                    wv_sb[:, kc, p * 256:(p + 1) * 256],
                                    kc == 0, kc == 1)
                        if p == 0:
                            nc.scalar.copy(v_sb[p][:, s0:s0 + 2, :], vt[:])
                        else:
                            nc.vector.tensor_copy(v_sb[p][:, s0:s0 + 2, :],
                                                  vt[:])
                    return run

                i = 0
                nt = len(tgs)
                for p in range(2):
                    thunks.append(kc_thunk(p, tgs[i % nt])); i += 1
                for h in range(4):
                    thunks.append(qt_thunk(h, tgs[i % nt])); i += 1
                for p in range(2):
                    for half in range(2):
                        thunks.append(vt_thunk(p, half, tgs[i % nt])); i += 1
                return thunks

            with tc.tile_pool(name=f"ps1_{rep}", bufs=1, space="PSUM") as ps1:
                pending = []
                for nq in range(4):
                    qs = slice(nq * 512, (nq + 1) * 512)
                    pb = [ps1.tile([P, 512], F32, tag=f"pb{g}",
                                   name=f"pb{g}_{nq}") for g in range(2)]
                    pa = [ps1.tile([P, 512], F32, tag=f"pa{g}",
                                   name=f"pa{g}_{nq}") for g in range(4)]
                    for kg in range(4):
                        ks = slice(4 * kg, 4 * (kg + 1))
                        if nq == 0:
                            if kg == 0:
                                # k=0 slices split across the two HWDGE
                                # queues so all three land ~2 issues deep
                                nc.scalar.dma_start(wrk_sb[:, 0:1, :],
                                                    w_rk[:, 0:1, :])
                                nc.sync.dma_start(wkvd_sb[:, 0:1, :],
                                                  w_kvd[:, 0:1, :])
                                nc.scalar.dma_start(wqd_sb[:, 0:1, :],
                                                    w_qd[:, 0:1, :])
                                for k1 in range(1, 4):
                                    s1 = slice(k1, k1 + 1)
                                    nc.scalar.dma_start(wrk_sb[:, s1, :],
                                                        w_rk[:, s1, :])
                                    nc.scalar.dma_start(wkvd_sb[:, s1, :],
                                                        w_kvd[:, s1, :])
                                    nc.scalar.dma_start(wqd_sb[:, s1, :],
                                                        w_qd[:, s1, :])
                            else:
                                nc.scalar.dma_start(wrk_sb[:, ks, :], w_rk[:, ks, :])
                                nc.scalar.dma_start(wkvd_sb[:, ks, :], w_kvd[:, ks, :])
                                nc.scalar.dma_start(wqd_sb[:, ks, :], w_qd[:, ks, :])
                        xtk = ph1.tile([P, 4, 512], BF16, tag="xtk",
                                       name="xtk", bufs=4)
                        x0 = nq * 16 + 4 * kg
                        if nq == 0 and kg == 0:
                            # split the first load so the k=0 matmuls can
                            # start after 128KB instead of 512KB
                            nc.sync.dma_start(xtk[:, 0:1, :], xTq[:, x0:x0 + 1, :])
                            nc.sync.dma_start(xtk[:, 1:2, :], xTq[:, x0 + 1:x0 + 2, :])
                            nc.sync.dma_start(xtk[:, 2:4, :], xTq[:, x0 + 2:x0 + 4, :])
                        else:
                            nc.sync.dma_start(xtk[:], xTq[:, x0:x0 + 4, :])
                        for j in range(4):
                            k = 4 * kg + j
                            start, stop = k == 0, k == 15
                            for g in range(2):
                                _mm(nc, pb[g][:],
                                    wrk_sb[:, k, g * 128:(g + 1) * 128],
                                    xtk[:, j, :], start, stop)
                            for g in range(4):
                                wsb, mi = wgl[g]
                                _mm(nc, pa[g][:],
                                    wsb[:, k, mi * 128:(mi + 1) * 128],
                                    xtk[:, j, :], start, stop)
                            # previous quarter's up-projection tiles, one
                            # per k-chunk: its latent drains finished
                            # chunks ago, and the >=2-chunk spacing between
                            # same-bank tiles hides each drain. Chunks
                            # 14/15 stay clean so this quarter's own
                            # drains aren't delayed at the boundary.
                            if 4 <= k <= 13 and pending:
                                pending.pop(0)()
                            # quarter-2's k-rope lands mid-quarter-3 so
                            # the phase boundary carries no DVE backlog
                            if nq == 3 and k == 11:
                                _rope_k_quarter(nc, rot, kT, kraw, cos_sb,
                                                sin_sb, 2, nc.vector)
                    if nq == 0:
                        # cos/sin + phase-2 weights first: quarter-0's
                        # rope and up-projections need them early
                        nc.scalar.dma_start(cos_sb[:], cosA[:, :])
                        nc.scalar.dma_start(sin_sb[:], sinB[:, :])
                        nc.scalar.dma_start(wkc_sb[:], w_kc[:, :, :])
                        nc.scalar.dma_start(wqc_sb[:], w_qc[:, :, :])
                        nc.scalar.dma_start(wv_sb[:], w_v[:, :, :])
                    elif nq == 1:
                        nc.scalar.dma_start(mask_sb[:], masks[:, :, :])
                        nc.scalar.dma_start(wo_sb[:], w_o[:, :, :])
                    # drains: alternate Act/DVE so neither serializes
                    nc.scalar.copy(kraw[0][:, qs], pb[0][:])
                    nc.vector.tensor_copy(kraw[1][:, qs], pb[1][:])
                    for g in range(4):
                        if g % 2 == 0:
                            nc.scalar.copy(targets[g][:, qs], pa[g][:])
                        else:
                            nc.vector.tensor_copy(targets[g][:, qs], pa[g][:])
                    # rope the PREVIOUS quarter's k (its drains + cos/sin
                    # have had a full quarter to land, so the DVE queue
                    # never blocks). kraw[p] rows [0:64]=h1, [64:128]=h0.
                    if 1 <= nq <= 2:
                        _rope_k_quarter(nc, rot, kT, kraw, cos_sb, sin_sb,
                                        nq - 1, nc.vector)
                    if nq < 3:
                        pending = pair_tiles(ps1, nq)

                # quarter-3's up-projections and ropes move into the SDPA
                # stream below: SDPA head 0 blocks 0-2 only touch
                # quarters 0-2, so they start the instant phase 1 ends

            def sdpa_head(ps3, h, extras=None, qb_list=None, pending=None,
                          flush=True, chunk_extras=None):
                # per-128-key-chunk score -> exp -> pv/rsum. Diagonal
                # chunks run FIRST with columns trimmed to the causal
                # range; off-diagonal chunks follow full-width, with
                # their probs pair-summed on DVE so the rsum ones-matmul
                # runs once per pair.
                p = h // 2
                hv = (h % 2) * 128

                def emit_norm(yp, rs, qs_):
                    # normalize straight out of PSUM (one DVE op fewer;
                    # ypv bufs=2 covers the slightly longer hold)
                    rcp = rot.tile([P, 512], F32, tag="rcp", name="rcp",
                                   bufs=2)
                    nc.vector.reciprocal_approx_fast(rcp[:], rs[:])
                    nc.vector.tensor_mul(y_sb[h][:, qs_], yp[:], rcp[:])

                if qb_list is None:
                    qb_list = range(NQB)
                for qb in qb_list:
                    q0 = qb * 512
                    qs = slice(q0, q0 + 512)
                    yps = ps3.tile([P, 512], F32, tag="ypv", name="ypv",
                                   bufs=2)
                    rsps = ps3.tile([P, 512], F32, tag="rs", name="rs",
                                    bufs=2)
                    # diagonal chunks first (trimmed), then off-diagonal
                    order = [4 * qb + dk for dk in range(4)] + \
                            list(range(4 * qb))
                    prev_prb = None
                    for idx, g in enumerate(order):
                        diag = g >= 4 * qb
                        off = (g - 4 * qb) * 128 if diag else 0
                        w = 512 - off
                        first = idx == 0
                        last = idx == len(order) - 1
                        scps = ps3.tile([P, 512], F32, tag="sc",
                                        name="sc", bufs=4)
                        nc.tensor.matmul(
                            scps[:, off:512],
                            kT[h][:, g * 128:(g + 1) * 128],
                            qT[h][:, q0 + off:q0 + 512],
                            start=True, stop=True,
                            skip_group_check=True)
                        prb = rot.tile([P, 512], BF16, tag="prb",
                                       name="prb", bufs=8)
                        nc.scalar.activation(
                            prb[:, off:512], scps[:, off:512], Exp,
                            bias=ebias_sb[:], scale=SCALE
                        )
                        if diag:
                            # alternate mask-mul between DVE and the
                            # otherwise-idle GpSimd engine
                            meng = nc.vector if g % 2 == 0 else nc.gpsimd
                            meng.tensor_mul(
                                prb[:, off:512], prb[:, off:512],
                                mask_sb[:, g - 4 * qb, off:512]
                            )
                        nc.tensor.matmul(
                            yps[:, off:512],
                            v_sb[p][:, g, hv:hv + 128],
                            prb[:, off:512],
                            start=first, stop=last,
                            skip_group_check=True)
                        if diag:
                            nc.tensor.matmul(
                                rsps[:, off:512], ones_sb[:],
                                prb[:, off:512],
                                start=first, stop=last,
                                skip_group_check=True)
                            prev_prb = None
                        elif prev_prb is None:
                            prev_prb = prb
                        else:
                            prs = rot.tile([P, 512], BF16, tag="prs",
                                           name="prs", bufs=3)
                            nc.vector.tensor_add(
                                prs[:], prev_prb[:], prb[:])
                            nc.tensor.matmul(
                                rsps[:], ones_sb[:], prs[:],
                                start=False, stop=last,
                                skip_group_check=True)
                            prev_prb = None
                        if idx == 1 and pending is not None:
                            # previous block's normalization, emitted two
                            # chunks into this block so the PE never waits
                            # on the Act queue at the boundary
                            emit_norm(*pending)
                            pending = None
                        # fine-grained deferred work (quarter-3 tiles),
                        # one item per chunk so psum drains hide in gaps
                        if chunk_extras and idx >= 1:
                            chunk_extras.pop(0)()
                    pending = (yps, rsps, qs)
                    # deferred quarter-3 rope work: each item ~2us of DVE,
                    # needed only by query-block 3 of its head
                    if extras:
                        extras.pop(0)()
                if flush and pending is not None:
                    emit_norm(*pending)
                    pending = None
                return pending

            with tc.tile_pool(name=f"ps3_{rep}", bufs=1, space="PSUM") as ps3:
                # head 0, blocks 0-2: needs only quarters 0-2, so the PE
                # rolls straight out of phase 1 with no boundary stall.
                # Quarter-3 up-projections ride the sc-tag psum rotation,
                # one tile per score chunk of blocks 1-2; ropes follow as
                # soon as their data exists.
                pend0 = sdpa_head(ps3, 0, qb_list=[0, 1, 2], flush=False)
                _rope_k_pair(nc, rot, kT, kraw, cos_sb, sin_sb, 3, 0,
                             nc.vector)                   # k-rope pair 0
                sdpa_extras = []
                p3 = pair_tiles(ps3, 3, tgs=("sc",),
                                deferred=sdpa_extras, tile_bufs=4)
                for t in p3[:3]:
                    t()
                sdpa_extras.pop(0)()                      # q-rope head 0
                for t in p3[3:]:
                    t()
                sdpa_extras.insert(1, lambda: _rope_k_pair(
                    nc, rot, kT, kraw, cos_sb, sin_sb, 3, 1, nc.vector))
                # flush head-0 block-2's norm under pair3's matmul cover,
                # then run head 1's blocks 0-2 (all inputs long-ready)
                # before head-0 block-3, so the deferred quarter-3 rope
                # chain on DVE finishes far ahead of the scores that
                # need it
                sdpa_head(ps3, 0, qb_list=[], pending=pend0, flush=True)
                pend1 = sdpa_head(ps3, 1, qb_list=[0, 1, 2],
                                  extras=sdpa_extras, flush=False)
                sdpa_head(ps3, 1, qb_list=[], pending=pend1, flush=True)
                sdpa_head(ps3, 0, qb_list=[3], extras=sdpa_extras)
                sdpa_head(ps3, 1, qb_list=[3], extras=sdpa_extras)
                for h in range(2, 4):
                    sdpa_head(ps3, h, extras=sdpa_extras)

            # -------- o_proj (all q blocks) --------
            # N=1024 matmuls (2 psum banks per tile): fewer instructions
            # and 2KB-per-partition output runs. Out tensor is
            # [qb2*16+oc, 128, 1024] so each DMA lands contiguous in
            # DRAM; DMAs alternate between the two HWDGE queues so the
            # out stream keeps pace with the PE and the kernel doesn't
            # end with a serial DMA drain.
            with tc.tile_pool(name=f"ps4_{rep}", bufs=1, space="PSUM") as ps4:
                for qb2 in range(2):
                    for oc in range(16):
                        opt_ = ps4.tile([P, 1024], F32, tag="opj", name="opj",
                                        bufs=3)
                        for q5 in range(2):
                            qs = slice(qb2 * 1024 + q5 * 512,
                                       qb2 * 1024 + (q5 + 1) * 512)
                            for hk in range(4):
                                _mm(nc, opt_[:, q5 * 512:(q5 + 1) * 512],
                                    wo_sb[:, hk, oc * 128:(oc + 1) * 128],
                                    y_sb[hk][:, qs], hk == 0, hk == 3)
                        osb = rot.tile([P, 1024], BF16, tag="osb", name="osb",
                                       bufs=4)
                        if oc % 2 == 0:
                            nc.vector.tensor_copy(osb[:], opt_[:])
                        else:
                            nc.scalar.copy(osb[:], opt_[:])
                        t = qb2 * 16 + oc
                        deng = nc.sync if oc % 2 == 0 else nc.scalar
                        deng.dma_start(out_r[t, :, :], osb[:])

      if bench_io:
          with tc.tile_pool(name="bo", bufs=1) as bo:
              bt = bo.tile([1, 128], BF16, tag="bt", name="bt")
              nc.sync.dma_start(bt[:], out[0, 0:1, 0:128])
              nc.sync.dma_start(outp[:, :], bt[:])

    nc.compile()
    return nc


_NC = None


def _get_nc():
    global _NC
    if _NC is None:
        _NC = build_program()
    return _NC


def _rope_tables():
    """cosA/sinB [128, S]: 32-row frequency pattern tiled 4x.
    sinB sign: rows [0:32] of each 64-block -> -sin, rows [32:64] -> +sin."""
    inv_freq = 1.0 / (BASE ** (np.arange(0, RD, 2, dtype=np.float32) / RD))  # [32]
    pos = np.arange(S, dtype=np.float32)
    ang = inv_freq[:, None] * pos[None, :]              # [32, S]
    cos1, sin1 = np.cos(ang), np.sin(ang)
    cosA = np.tile(cos1, (4, 1))                        # [128, S]
    sinB = np.concatenate([-sin1, sin1, -sin1, sin1], axis=0)
    return cosA.astype(_bf), sinB.astype(_bf)


def _mask_tiles():
    """masks[d][k, q] = 1.0 if q >= d*128 + k else 0 (bf16, [4,128,512])."""
    k = np.arange(P)[:, None]
    q = np.arange(512)[None, :]
    m = np.stack([(q >= d * 128 + k) for d in range(4)]).astype(np.float32)
    return np.ascontiguousarray(m.transpose(1, 0, 2)).astype(_bf)


def _prep_core_inputs(c, x, W_kv_d, W_q_d, W_k_u, W_q_u, W_v_u, W_rope_k, W_rope_q,
                      W_o, cosA, sinB, masks):
    b = c // 4
    hg = c % 4
    heads = [4 * hg + j for j in range(HEADS_PER_CORE)]

    def tile_pmaj(w):
        # [ko*128, m] -> [128, ko, m] partition-major for contiguous DMA
        ko = w.shape[0] // P
        return np.ascontiguousarray(
            w.reshape(ko, P, w.shape[1]).transpose(1, 0, 2))

    xT = np.ascontiguousarray(x[b].T).astype(_bf)                  # [H, S]
    # xTq[p, nq*16+k, c] = xT[k*128+p, nq*512+c]
    xTq = np.ascontiguousarray(
        xT.reshape(16, P, 4, 512).transpose(1, 2, 0, 3).reshape(P, 64, 512))
    w_kvd = tile_pmaj(np.ascontiguousarray(W_kv_d.T).astype(_bf))
    w_qd = tile_pmaj(np.ascontiguousarray(W_q_d.T).astype(_bf))

    # w_rk: per pair, rows [h1 rope dims | h0 rope dims], then transpose
    blocks = []
    for p in range(2):
        g0, g1 = heads[2 * p], heads[2 * p + 1]
        blocks.append(W_rope_k[g1 * RD:(g1 + 1) * RD, :])
        blocks.append(W_rope_k[g0 * RD:(g0 + 1) * RD, :])
    w_rk = tile_pmaj(np.ascontiguousarray(np.concatenate(blocks, axis=0).T).astype(_bf))

    # w_qc: per local head 128 cols: even -> [content|rope], odd -> [rope|content]
    cols = []
    for j, g in enumerate(heads):
        c_blk = W_q_u[g * RD:(g + 1) * RD, :].T       # [LAT, 64]
        r_blk = W_rope_q[g * RD:(g + 1) * RD, :].T    # [LAT, 64]
        cols.extend([c_blk, r_blk] if j % 2 == 0 else [r_blk, c_blk])
    w_qc = tile_pmaj(np.ascontiguousarray(np.concatenate(cols, axis=1)).astype(_bf))

    # w_kc: per pair 128 cols: [h0 content | h1 content]
    cols = []
    for p in range(2):
        g0, g1 = heads[2 * p], heads[2 * p + 1]
        cols.append(W_k_u[g0 * RD:(g0 + 1) * RD, :].T)
        cols.append(W_k_u[g1 * RD:(g1 + 1) * RD, :].T)
    w_kc = tile_pmaj(np.ascontiguousarray(np.concatenate(cols, axis=1)).astype(_bf))

    # w_v: per pair 256 cols: [h0 v dims | h1 v dims]
    cols = []
    for p in range(2):
        g0, g1 = heads[2 * p], heads[2 * p + 1]
        cols.append(W_v_u[g0 * HD:(g0 + 1) * HD, :].T)
        cols.append(W_v_u[g1 * HD:(g1 + 1) * HD, :].T)
    w_v = tile_pmaj(np.ascontiguousarray(np.concatenate(cols, axis=1)).astype(_bf))

    d0 = heads[0] * HD
    w_o = tile_pmaj(np.ascontiguousarray(W_o[:, d0:d0 + 512].T).astype(_bf))

    return {
        "xTq": xTq, "w_kvd": w_kvd, "w_qd": w_qd, "w_rk": w_rk, "w_qc": w_qc,
        "w_kc": w_kc, "w_v": w_v, "w_o": w_o, "cosA": cosA, "sinB": sinB,
        "masks": masks,
    }


def make_in_maps(inputs):
    x = np.asarray(inputs["hidden_states"], dtype=np.float32)
    ws = {k: np.asarray(inputs[k], dtype=np.float32)
          for k in ("W_kv_d", "W_q_d", "W_k_u", "W_q_u", "W_v_u", "W_rope_k",
                    "W_rope_q", "W_o")}
    cosA, sinB = _rope_tables()
    masks = _mask_tiles()
    return [
        _prep_core_inputs(c, x, ws["W_kv_d"], ws["W_q_d"], ws["W_k_u"],
                          ws["W_q_u"], ws["W_v_u"], ws["W_rope_k"],
                          ws["W_rope_q"], ws["W_o"], cosA, sinB, masks)
        for c in range(N_CORES)
    ]


def assemble(results):
    """results: list of 8 dicts with 'out' [32, 128, 1024] bf16 partials.
    Tile t = qb2*16 + oc holds rows oc*128:(oc+1)*128, cols
    qb2*1024:(qb2+1)*1024 of the [H, S] transposed partial."""
    full = np.empty((B, S, H), dtype=np.float32)
    for b in range(B):
        acc = results[4 * b]["out"].astype(np.float32)
        for g in range(1, 4):
            acc = acc + results[4 * b + g]["out"]
        # [2, 16, 128, 1024] -> [H, S]
        ht = acc.reshape(2, 16, P, 1024).transpose(1, 2, 0, 3).reshape(H, S)
        full[b] = ht.T
    return full


def kernel(**inputs):
    nc = _get_nc()
    in_maps = make_in_maps(inputs)
    res = run_bass_kernel_spmd(nc, in_maps, core_ids=list(range(N_CORES)))
    return assemble(res.results)



# revision 70
# speedup vs baseline: 1.0016x; 1.0016x over previous
"""MLA (multi-head latent attention) Trainium2 kernel.

Sharding: 8 cores = 2 (batch) x 4 (head groups of 4 heads).
Each core computes, for its batch b and heads [4g, 4g+4):
  latents kv_d/q_d (replicated within the batch group), per-head
  up-projections + RoPE, causal SDPA, and a partial o_proj
  out_core[o, q] = sum_{d in core's 512 head-dims} W_o[o, d] * y[d, q].
Host sums the 4 partials per batch (the all-reduce step of the hint,
performed at unshard time) and transposes to [S, H].

All matmuls run in bf16 with fp32 PSUM accumulation.

Phase 1 is fused: one pass over xT (stored k-chunk/quarter-major as
[128, 64, 512]) computes rope-k and both latents per S-quarter, using
6 PSUM banks, so xT is read once and the PE never waits on a second
sweep. PSUM drains alternate Act/DVE so neither engine serializes the
pipeline. SDPA runs per 128-key chunk with deep PSUM buffering.
"""

import numpy as np
import ml_dtypes

import concourse.bass as bass
import concourse.mybir as mybir
import concourse.tile as tile
from concourse import bacc
from concourse._compat import get_trn_type
from concourse.bass_utils import run_bass_kernel_spmd

H = 2048
NH = 16
HD = 128           # head dim
RD = 64            # rotary dim
RH = 32            # rotary half
LAT = 256
B = 2
S = 2048
BASE = 10000.0
N_CORES = 8
HEADS_PER_CORE = 4
P = 128
NQB = S // 512     # 4 query blocks of 512
NKI = S // 128     # 16 key chunks of 128
SCALE = 1.0 / np.sqrt(float(HD))
EXP_BIAS = -4.0

BF16 = mybir.dt.bfloat16
F32 = mybir.dt.float32
_bf = ml_dtypes.bfloat16


def _mm(nc, out, lhsT, rhs, start, stop):
    nc.tensor.matmul(out, lhsT, rhs, start=start, stop=stop)


def _rope_k_pair(nc, rot, kT, kraw, cos_sb, sin_sb, nq, p, eng):
    """Apply rope to quarter nq of kraw for pair p on engine `eng`.
    Swaps go via SBUF-SBUF DMA on the sync queue."""
    qs = slice(nq * 512, (nq + 1) * 512)
    h0, h1 = 2 * p, 2 * p + 1
    swq = rot.tile([P, 512], BF16, tag="swq", name="swq", bufs=3)
    nc.sync.dma_start(swq[0:32, :], kraw[p][32:64, qs])
    nc.sync.dma_start(swq[32:64, :], kraw[p][0:32, qs])
    nc.sync.dma_start(swq[64:96, :], kraw[p][96:128, qs])
    nc.sync.dma_start(swq[96:128, :], kraw[p][64:96, qs])
    eng.tensor_mul(swq[:], swq[:], sin_sb[:, qs])
    eng.tensor_mul(kT[h1][0:64, qs], kraw[p][0:64, qs], cos_sb[0:64, qs])
    eng.tensor_add(kT[h1][0:64, qs], kT[h1][0:64, qs], swq[0:64, :])
    eng.tensor_mul(kT[h0][64:128, qs], kraw[p][64:128, qs],
                   cos_sb[64:128, qs])
    eng.tensor_add(kT[h0][64:128, qs], kT[h0][64:128, qs],
                   swq[64:128, :])


def _rope_k_quarter(nc, rot, kT, kraw, cos_sb, sin_sb, nq, eng):
    for p in range(2):
        _rope_k_pair(nc, rot, kT, kraw, cos_sb, sin_sb, nq, p, eng)


def build_program(nrep=1, bench_io=False):
    nc = bacc.Bacc(
        get_trn_type() or "TRN2",
        target_bir_lowering=False,
        debug=False,
        num_devices=N_CORES,
    )

    if bench_io:
        dummy = nc.declare_dram_parameter("bdummy", [1, 128], F32, isOutput=False)
        xTq = nc.dram_tensor("xTq", [P, 64, 512], BF16)
        w_kvd = nc.dram_tensor("w_kvd", [P, 16, LAT], BF16)
        w_qd = nc.dram_tensor("w_qd", [P, 16, LAT], BF16)
        w_rk = nc.dram_tensor("w_rk", [P, 16, 256], BF16)
        w_qc = nc.dram_tensor("w_qc", [P, 2, 512], BF16)
        w_kc = nc.dram_tensor("w_kc", [P, 2, 256], BF16)
        w_v = nc.dram_tensor("w_v", [P, 2, 512], BF16)
        w_o = nc.dram_tensor("w_o", [P, 4, H], BF16)
        cosA = nc.dram_tensor("cosA", [P, S], BF16)
        sinB = nc.dram_tensor("sinB", [P, S], BF16)
        masks = nc.dram_tensor("masks", [P, 4, 512], BF16)
        out = nc.dram_tensor("outs", [32, P, 1024], BF16)
        outp = nc.declare_dram_parameter("out", [1, 128], BF16, isOutput=True)
    else:
        xTq = nc.declare_dram_parameter("xTq", [P, 64, 512], BF16, isOutput=False)
        w_kvd = nc.declare_dram_parameter("w_kvd", [P, 16, LAT], BF16, isOutput=False)
        w_qd = nc.declare_dram_parameter("w_qd", [P, 16, LAT], BF16, isOutput=False)
        w_rk = nc.declare_dram_parameter("w_rk", [P, 16, 256], BF16, isOutput=False)
        w_qc = nc.declare_dram_parameter("w_qc", [P, 2, 512], BF16, isOutput=False)
        w_kc = nc.declare_dram_parameter("w_kc", [P, 2, 256], BF16, isOutput=False)
        w_v = nc.declare_dram_parameter("w_v", [P, 2, 512], BF16, isOutput=False)
        w_o = nc.declare_dram_parameter("w_o", [P, 4, H], BF16, isOutput=False)
        cosA = nc.declare_dram_parameter("cosA", [P, S], BF16, isOutput=False)
        sinB = nc.declare_dram_parameter("sinB", [P, S], BF16, isOutput=False)
        masks = nc.declare_dram_parameter("masks", [P, 4, 512], BF16, isOutput=False)
        out = nc.declare_dram_parameter("out", [32, P, 1024], BF16, isOutput=True)

    Exp = mybir.ActivationFunctionType.Exp

    scratch = (nc.dram_tensor("scratch", [32, P, 1024], BF16) if nrep > 1 else None)

    with tile.TileContext(nc) as tc:
      for rep in range(nrep):
        out_r = out if rep == nrep - 1 else scratch
        with (
            tc.tile_pool(name=f"wpool{rep}", bufs=1) as wpool,
            tc.tile_pool(name=f"main1_{rep}", bufs=1) as main1,
            tc.tile_pool(name=f"main2_{rep}", bufs=1) as main2,
            tc.tile_pool(name=f"ph1_{rep}", bufs=1) as ph1,
            tc.tile_pool(name=f"rot{rep}", bufs=3) as rot,
        ):
            # -------- persistent small tensors (DMAs deferred) --------
            cos_sb = wpool.tile([P, S], BF16, tag="cos", name="cos")
            sin_sb = wpool.tile([P, S], BF16, tag="sin", name="sin")
            mask_sb = wpool.tile([P, 4, 512], BF16, tag="mask", name="mask")
            ones_sb = wpool.tile([P, P], BF16, tag="ones", name="ones")
            nc.gpsimd.memset(ones_sb[:], 1.0)
            ebias_sb = wpool.tile([P, 1], F32, tag="ebias", name="ebias")
            nc.gpsimd.memset(ebias_sb[:], EXP_BIAS)
            wo_sb = wpool.tile([P, 4, H], BF16, tag="wo", name="wo")
            wqc_sb = wpool.tile([P, 2, 512], BF16, tag="wqc", name="wqc")
            wkc_sb = wpool.tile([P, 2, 256], BF16, tag="wkc", name="wkc")
            wv_sb = wpool.tile([P, 2, 512], BF16, tag="wv", name="wv")

            # -------- phase-1 outputs (latents + raw rope-k), bf16 --------
            kvd_sb = [main1.tile([P, S], BF16, tag=f"kvd{m}", name=f"kvd{m}")
                      for m in range(2)]
            qd_sb = [main1.tile([P, S], BF16, tag=f"qd{m}", name=f"qd{m}")
                     for m in range(2)]
            kraw = [main1.tile([P, S], BF16, tag=f"kraw{p}", name=f"kraw{p}")
                    for p in range(2)]

            # phase-1 weights, loaded JIT in k-groups during quarter 0
            wkvd_sb = ph1.tile([P, 16, LAT], BF16, tag="wkvd", name="wkvd")
            wqd_sb = ph1.tile([P, 16, LAT], BF16, tag="wqd", name="wqd")
            wrk_sb = ph1.tile([P, 16, 256], BF16, tag="wrk", name="wrk")

            # -------- per-head q/k tiles (dims on partitions), v, y --------
            # even head h: rows [0:64] content, [64:128] rope
            # odd  head h: rows [0:64] rope,    [64:128] content
            qT = [main2.tile([P, S], BF16, tag=f"qT{h}", name=f"qT{h}")
                  for h in range(4)]
            kT = [main2.tile([P, S], BF16, tag=f"kT{h}", name=f"kT{h}")
                  for h in range(4)]
            v_sb = [main2.tile([P, NKI, 256], BF16, tag=f"v{p}", name=f"v{p}")
                    for p in range(2)]
            y_sb = [main2.tile([P, S], BF16, tag=f"y{h}", name=f"y{h}")
                    for h in range(4)]

            # -------- phase 1 (fused): per S-quarter, one xT sweep feeds
            # rope-k (2 pair-groups) + latents (kvd x2, qd x2): 6 banks.
            # Phase 2 (k/q/v up-projections + both ropes) is folded in:
            # quarter nq's pair work is emitted one quarter later (kg==1
            # boundary), riding the PE-bound stream's Act/DVE slack, so
            # the old elementwise-bound pair phases disappear. ----
            wgl = [(wkvd_sb, 0), (wkvd_sb, 1), (wqd_sb, 0), (wqd_sb, 1)]
            targets = [kvd_sb[0], kvd_sb[1], qd_sb[0], qd_sb[1]]

            # -------- HAM warm-up --------
            # The PE sits idle through the ~6.5us framework preamble and
            # first-weight DMA wait, so the HAM clock-gate holds 1.2GHz
            # for the first ~5us of real matmuls. A dependency-free dummy
            # matmul stream (ones x whatever SBUF holds; result never
            # read) fills that dead window and trips the 4096-cycle
            # busy-window to 2.4GHz before the real stream starts.
            with tc.tile_pool(name=f"warm{rep}", bufs=1,
                              space="PSUM") as wps:
                wt = wps.tile([P, 512], F32, tag="warm", name="warm")
                for _ in range(12):
                    nc.tensor.matmul(wt[:], ones_sb[:], wo_sb[:, 0, 0:512],
                                     start=True, stop=True,
                                     skip_group_check=True)

            def q_rope_quarter(h, nq):
                # rope on q for head h, quarter nq (DVE; swaps via
                # SBUF-SBUF DMA on the scalar queue)
                r = 64 if h % 2 == 0 else 0
                qs = slice(nq * 512, (nq + 1) * 512)
                swp = rot.tile([P, 512], BF16, tag="swp", name="swp", bufs=3)
                nc.scalar.dma_start(swp[r:r + 32, :], qT[h][r + 32:r + 64, qs])
                nc.scalar.dma_start(swp[r + 32:r + 64, :], qT[h][r:r + 32, qs])
                nc.vector.tensor_mul(
                    qT[h][r:r + 64, qs], qT[h][r:r + 64, qs],
                    cos_sb[r:r + 64, qs]
                )
                nc.vector.tensor_mul(
                    swp[r:r + 64, :], swp[r:r + 64, :], sin_sb[r:r + 64, qs]
                )
                nc.vector.tensor_add(
                    qT[h][r:r + 64, qs], qT[h][r:r + 64, qs], swp[r:r + 64, :]
                )

            def pair_tiles(ps1, nq, tgs=("px", "py"), deferred=None,
                           tile_bufs=1):
                # Phase-2 work for column-quarter nq as 10 independent
                # thunks (2 k-content, 4 q+rope, 4 v), each one psum tile
                # on the ps1 pool's two spare banks (tags px/py). The
                # caller interleaves them between ph1 k-chunks so drains
                # always finish in the gaps.
                ns = slice(nq * 512, (nq + 1) * 512)
                thunks = []

                def kc_thunk(p, tag):
                    def run():
                        h0, h1 = 2 * p, 2 * p + 1
                        pt = ps1.tile([P, 512], F32, tag=tag,
                                      name=f"kc{p}_{nq}", bufs=tile_bufs)
                        for kc in range(2):
                            _mm(nc, pt[:],
                                wkc_sb[:, kc, p * 128:(p + 1) * 128],
                                kvd_sb[kc][:, ns], kc == 0, kc == 1)
                        if p == 0:
                            nc.scalar.copy(kT[h0][0:64, ns], pt[0:64, :])
                            nc.vector.tensor_copy(kT[h1][64:128, ns],
                                                  pt[64:128, :])
                        else:
                            nc.vector.tensor_copy(kT[h0][0:64, ns],
                                                  pt[0:64, :])
                            nc.scalar.copy(kT[h1][64:128, ns],
                                           pt[64:128, :])
                    return run

                def qt_thunk(h, tag):
                    def run():
                        qt = ps1.tile([P, 512], F32, tag=tag,
                                      name=f"qt{h}_{nq}", bufs=tile_bufs)
                        for kc in range(2):
                            _mm(nc, qt[:],
                                wqc_sb[:, kc, h * 128:(h + 1) * 128],
                                qd_sb[kc][:, ns], kc == 0, kc == 1)
                        if h % 2 == 0:
                            nc.scalar.copy(qT[h][:, ns], qt[:])
                        else:
                            nc.vector.tensor_copy(qT[h][:, ns], qt[:])
                        if deferred is None:
                            q_rope_quarter(h, nq)
                        else:
                            deferred.append(
                                lambda h=h: q_rope_quarter(h, nq))
                    return run

                def vt_thunk(p, half, tag):
                    def run():
                        s0 = 4 * nq + 2 * half
                        vt = ps1.tile([P, 512], F32, tag=tag,
                                      name=f"vt{p}_{nq}_{half}", bufs=tile_bufs)
                        for c2 in range(2):
                            for kc in range(2):
                                _mm(nc, vt[:, c2 * 256:(c2 + 1) * 256],
                                    kvd_sb[kc][:, (s0 + c2) * 128:
                                               (s0 + c2 + 1) * 128],
                                    wv_sb[:, kc, p * 256:(p + 1) * 256],
                                    kc == 0, kc == 1)
                        if p == 0:
                            nc.scalar.copy(v_sb[p][:, s0:s0 + 2, :], vt[:])
                        else:
                            nc.vector.tensor_copy(v_sb[p][:, s0:s0 + 2, :],
                                                  vt[:])
                    return run

                i = 0
                nt = len(tgs)
                for p in range(2):
                    thunks.append(kc_thunk(p, tgs[i % nt])); i += 1
                for h in range(4):
                    thunks.append(qt_thunk(h, tgs[i % nt])); i += 1
                for p in range(2):
                    for half in range(2):
                        thunks.append(vt_thunk(p, half, tgs[i % nt])); i += 1
                return thunks

            with tc.tile_pool(name=f"ps1_{rep}", bufs=1, space="PSUM") as ps1:
                pending = []
                for nq in range(4):
                    qs = slice(nq * 512, (nq + 1) * 512)
                    pb = [ps1.tile([P, 512], F32, tag=f"pb{g}",
                                   name=f"pb{g}_{nq}") for g in range(2)]
                    pa = [ps1.tile([P, 512], F32, tag=f"pa{g}",
                                   name=f"pa{g}_{nq}") for g in range(4)]
                    for kg in range(4):
                        ks = slice(4 * kg, 4 * (kg + 1))
                        if nq == 0:
                            if kg == 0:
                                # k=0 slices split across the two HWDGE
                                # queues so all three land ~2 issues deep
                                nc.scalar.dma_start(wrk_sb[:, 0:1, :],
                                                    w_rk[:, 0:1, :])
                                nc.sync.dma_start(wkvd_sb[:, 0:1, :],
                                                  w_kvd[:, 0:1, :])
                                nc.scalar.dma_start(wqd_sb[:, 0:1, :],
                                                    w_qd[:, 0:1, :])
                                for k1 in range(1, 4):
                                    s1 = slice(k1, k1 + 1)
                                    nc.scalar.dma_start(wrk_sb[:, s1, :],
                                                        w_rk[:, s1, :])
                                    nc.scalar.dma_start(wkvd_sb[:, s1, :],
                                                        w_kvd[:, s1, :])
                                    nc.scalar.dma_start(wqd_sb[:, s1, :],
                                                        w_qd[:, s1, :])
                            else:
                                nc.scalar.dma_start(wrk_sb[:, ks, :], w_rk[:, ks, :])
                                nc.scalar.dma_start(wkvd_sb[:, ks, :], w_kvd[:, ks, :])
                                nc.scalar.dma_start(wqd_sb[:, ks, :], w_qd[:, ks, :])
                        xtk = ph1.tile([P, 4, 512], BF16, tag="xtk",
                                       name="xtk", bufs=4)
                        x0 = nq * 16 + 4 * kg
                        if nq == 0 and kg == 0:
                            # split the first load so the k=0 matmuls can
                            # start after 128KB instead of 512KB
                            nc.sync.dma_start(xtk[:, 0:1, :], xTq[:, x0:x0 + 1, :])
                            nc.sync.dma_start(xtk[:, 1:2, :], xTq[:, x0 + 1:x0 + 2, :])
                            nc.sync.dma_start(xtk[:, 2:4, :], xTq[:, x0 + 2:x0 + 4, :])
                        else:
                            nc.sync.dma_start(xtk[:], xTq[:, x0:x0 + 4, :])
                        for j in range(4):
                            k = 4 * kg + j
                            start, stop = k == 0, k == 15
                            for g in range(2):
                                _mm(nc, pb[g][:],
                                    wrk_sb[:, k, g * 128:(g + 1) * 128],
                                    xtk[:, j, :], start, stop)
                            for g in range(4):
                                wsb, mi = wgl[g]
                                _mm(nc, pa[g][:],
                                    wsb[:, k, mi * 128:(mi + 1) * 128],
                                    xtk[:, j, :], start, stop)
                            # previous quarter's up-projection tiles, one
                            # per k-chunk: its latent drains finished
                            # chunks ago, and the >=2-chunk spacing between
                            # same-bank tiles hides each drain. Chunks
                            # 14/15 stay clean so this quarter's own
                            # drains aren't delayed at the boundary.
                            if 4 <= k <= 13 and pending:
                                pending.pop(0)()
                            # quarter-2's k-rope lands mid-quarter-3 so
                            # the phase boundary carries no DVE backlog
                            if nq == 3 and k == 11:
                                _rope_k_quarter(nc, rot, kT, kraw, cos_sb,
                                                sin_sb, 2, nc.vector)
                    if nq == 0:
                        # cos/sin + phase-2 weights first: quarter-0's
                        # rope and up-projections need them early
                        nc.scalar.dma_start(cos_sb[:], cosA[:, :])
                        nc.scalar.dma_start(sin_sb[:], sinB[:, :])
                        nc.scalar.dma_start(wkc_sb[:], w_kc[:, :, :])
                        nc.scalar.dma_start(wqc_sb[:], w_qc[:, :, :])
                        nc.scalar.dma_start(wv_sb[:], w_v[:, :, :])
                    elif nq == 1:
                        nc.scalar.dma_start(mask_sb[:], masks[:, :, :])
                        nc.scalar.dma_start(wo_sb[:], w_o[:, :, :])
                    # drains: alternate Act/DVE so neither serializes
                    nc.scalar.copy(kraw[0][:, qs], pb[0][:])
                    nc.vector.tensor_copy(kraw[1][:, qs], pb[1][:])
                    for g in range(4):
                        if g % 2 == 0:
                            nc.scalar.copy(targets[g][:, qs], pa[g][:])
                        else:
                            nc.vector.tensor_copy(targets[g][:, qs], pa[g][:])
                    # rope the PREVIOUS quarter's k (its drains + cos/sin
                    # have had a full quarter to land, so the DVE queue
                    # never blocks). kraw[p] rows [0:64]=h1, [64:128]=h0.
                    if 1 <= nq <= 2:
                        _rope_k_quarter(nc, rot, kT, kraw, cos_sb, sin_sb,
                                        nq - 1, nc.vector)
                    if nq < 3:
                        pending = pair_tiles(ps1, nq)

                # quarter-3's up-projections and ropes move into the SDPA
                # stream below: SDPA head 0 blocks 0-2 only touch
                # quarters 0-2, so they start the instant phase 1 ends

            def sdpa_head(ps3, h, extras=None, qb_list=None, pending=None,
                          flush=True, chunk_extras=None):
                # per-128-key-chunk score -> exp -> pv/rsum. Diagonal
                # chunks run FIRST with columns trimmed to the causal
                # range; off-diagonal chunks follow full-width, with
                # their probs pair-summed on DVE so the rsum ones-matmul
                # runs once per pair.
                p = h // 2
                hv = (h % 2) * 128

                def emit_norm(yp, rs, qs_):
                    # normalize straight out of PSUM (one DVE op fewer;
                    # ypv bufs=2 covers the slightly longer hold)
                    rcp = rot.tile([P, 512], F32, tag="rcp", name="rcp",
                                   bufs=2)
                    nc.vector.reciprocal_approx_fast(rcp[:], rs[:])
                    nc.vector.tensor_mul(y_sb[h][:, qs_], yp[:], rcp[:])

                if qb_list is None:
                    qb_list = range(NQB)
                for qb in qb_list:
                    q0 = qb * 512
                    qs = slice(q0, q0 + 512)
                    yps = ps3.tile([P, 512], F32, tag="ypv", name="ypv",
                                   bufs=2)
                    rsps = ps3.tile([P, 512], F32, tag="rs", name="rs",
                                    bufs=2)
                    # diagonal chunks first (trimmed), then off-diagonal
                    order = [4 * qb + dk for dk in range(4)] + \
                            list(range(4 * qb))
                    prev_prb = None
                    for idx, g in enumerate(order):
                        diag = g >= 4 * qb
                        off = (g - 4 * qb) * 128 if diag else 0
                        w = 512 - off
                        first = idx == 0
                        last = idx == len(order) - 1
                        scps = ps3.tile([P, 512], F32, tag="sc",
                                        name="sc", bufs=4)
                        nc.tensor.matmul(
                            scps[:, off:512],
                            kT[h][:, g * 128:(g + 1) * 128],
                            qT[h][:, q0 + off:q0 + 512],
                            start=True, stop=True,
                            skip_group_check=True)
                        prb = rot.tile([P, 512], BF16, tag="prb",
                                       name="prb", bufs=8)
                        nc.scalar.activation(
                            prb[:, off:512], scps[:, off:512], Exp,
                            bias=ebias_sb[:], scale=SCALE
                        )
                        if diag:
                            # alternate mask-mul between DVE and the
                            # otherwise-idle GpSimd engine
                            meng = nc.vector if g % 2 == 0 else nc.gpsimd
                            meng.tensor_mul(
                                prb[:, off:512], prb[:, off:512],
                                mask_sb[:, g - 4 * qb, off:512]
                            )
                        nc.tensor.matmul(
                            yps[:, off:512],
                            v_sb[p][:, g, hv:hv + 128],
                            prb[:, off:512],
                            start=first, stop=last,
                            skip_group_check=True)
                        if diag:
                            nc.tensor.matmul(
                                rsps[:, off:512], ones_sb[:],
                                prb[:, off:512],
                                start=first, stop=last,
                                skip_group_check=True)
                            prev_prb = None
                        elif prev_prb is None:
                            prev_prb = prb
                        else:
                            prs = rot.tile([P, 512], BF16, tag="prs",
                                           name="prs", bufs=3)
                            nc.vector.tensor_add(
                                prs[:], prev_prb[:], prb[:])
                            nc.tensor.matmul(
                                rsps[:], ones_sb[:], prs[:],
                                start=False, stop=last,
                                skip_group_check=True)
                            prev_prb = None
                        if idx == 1 and pending is not None:
                            # previous block's normalization, emitted two
                            # chunks into this block so the PE never waits
                            # on the Act queue at the boundary
                            emit_norm(*pending)
                            pending = None
                        # fine-grained deferred work (quarter-3 tiles),
                        # one item per chunk so psum drains hide in gaps
                        if chunk_extras and idx >= 1:
                            chunk_extras.pop(0)()
                    pending = (yps, rsps, qs)
                    # deferred quarter-3 rope work: each item ~2us of DVE,
                    # needed only by query-block 3 of its head
                    if extras:
                        extras.pop(0)()
                if flush and pending is not None:
                    emit_norm(*pending)
                    pending = None
                return pending

            with tc.tile_pool(name=f"ps3_{rep}", bufs=1, space="PSUM") as ps3:
                # head 0, blocks 0-2: needs only quarters 0-2, so the PE
                # rolls straight out of phase 1 with no boundary stall.
                # Quarter-3 up-projections ride the sc-tag psum rotation,
                # one tile per score chunk of blocks 1-2; ropes follow as
                # soon as their data exists.
                pend0 = sdpa_head(ps3, 0, qb_list=[0, 1, 2], flush=False)
                _rope_k_pair(nc, rot, kT, kraw, cos_sb, sin_sb, 3, 0,
                             nc.vector)                   # k-rope pair 0
                sdpa_extras = []
                p3 = pair_tiles(ps3, 3, tgs=("sc",),
                                deferred=sdpa_extras, tile_bufs=4)
                for t in p3[:3]:
                    t()
                sdpa_extras.pop(0)()                      # q-rope head 0
                for t in p3[3:]:
                    t()
                sdpa_extras.insert(1, lambda: _rope_k_pair(
                    nc, rot, kT, kraw, cos_sb, sin_sb, 3, 1, nc.vector))
                # flush head-0 block-2's norm under pair3's matmul cover,
                # then run head 1's blocks 0-2 (all inputs long-ready)
                # before head-0 block-3, so the deferred quarter-3 rope
                # chain on DVE finishes far ahead of the scores that
                # need it
                sdpa_head(ps3, 0, qb_list=[], pending=pend0, flush=True)
                pend1 = sdpa_head(ps3, 1, qb_list=[0, 1, 2],
                                  extras=sdpa_extras, flush=False)
                sdpa_head(ps3, 1, qb_list=[], pending=pend1, flush=True)
                sdpa_head(ps3, 0, qb_list=[3], extras=sdpa_extras)
                sdpa_head(ps3, 1, qb_list=[3], extras=sdpa_extras)
                for h in range(2, 4):
                    sdpa_head(ps3, h, extras=sdpa_extras)
                # first two o_proj oc-tiles ride the sc banks (freed by
                # exp long before the ypv/rs banks), so the PE rolls
                # straight out of head 3 while the ps3->ps4 pool handoff
                # drains behind it
                for oc in range(2):
                    for q5 in range(2):
                        pt = ps3.tile([P, 512], F32, tag="sc",
                                      name=f"opre{oc}{q5}", bufs=4)
                        for hk in range(4):
                            _mm(nc, pt[:],
                                wo_sb[:, hk, oc * 128:(oc + 1) * 128],
                                y_sb[hk][:, q5 * 512:(q5 + 1) * 512],
                                hk == 0, hk == 3)
                        osb2 = rot.tile([P, 512], BF16, tag="prs",
                                        name="osb2", bufs=3)
                        if q5 == 0:
                            nc.vector.tensor_copy(osb2[:], pt[:])
                            nc.sync.dma_start(
                                out_r[oc, :, 0:512], osb2[:])
                        else:
                            nc.scalar.copy(osb2[:], pt[:])
                            nc.scalar.dma_start(
                                out_r[oc, :, 512:1024], osb2[:])

            # -------- o_proj (all q blocks) --------
            # N=1024 matmuls (2 psum banks per tile): fewer instructions
            # and 2KB-per-partition output runs. Out tensor is
            # [qb2*16+oc, 128, 1024] so each DMA lands contiguous in
            # DRAM; DMAs alternate between the two HWDGE queues so the
            # out stream keeps pace with the PE and the kernel doesn't
            # end with a serial DMA drain.
            with tc.tile_pool(name=f"ps4_{rep}", bufs=1, space="PSUM") as ps4:
                for qb2 in range(2):
                    for oc in range(2 if qb2 == 0 else 0, 16):
                        opt_ = ps4.tile([P, 1024], F32, tag="opj", name="opj",
                                        bufs=3)
                        for q5 in range(2):
                            qs = slice(qb2 * 1024 + q5 * 512,
                                       qb2 * 1024 + (q5 + 1) * 512)
                            for hk in range(4):
                                _mm(nc, opt_[:, q5 * 512:(q5 + 1) * 512],
                                    wo_sb[:, hk, oc * 128:(oc + 1) * 128],
                                    y_sb[hk][:, qs], hk == 0, hk == 3)
                        osb = rot.tile([P, 1024], BF16, tag="osb", name="osb",
                                       bufs=4)
                        if oc % 2 == 0:
                            nc.vector.tensor_copy(osb[:], opt_[:])
                        else:
                            nc.scalar.copy(osb[:], opt_[:])
                        t = qb2 * 16 + oc
                        deng = nc.sync if oc % 2 == 0 else nc.scalar
                        deng.dma_start(out_r[t, :, :], osb[:])

      if bench_io:
          with tc.tile_pool(name="bo", bufs=1) as bo:
              bt = bo.tile([1, 128], BF16, tag="bt", name="bt")
              nc.sync.dma_start(bt[:], out[0, 0:1, 0:128])
              nc.sync.dma_start(outp[:, :], bt[:])

    nc.compile()
    return nc


_NC = None


def _get_nc():
    global _NC
    if _NC is None:
        _NC = build_program()
    return _NC


def _rope_tables():
    """cosA/sinB [128, S]: 32-row frequency pattern tiled 4x.
    sinB sign: rows [0:32] of each 64-block -> -sin, rows [32:64] -> +sin."""
    inv_freq = 1.0 / (BASE ** (np.arange(0, RD, 2, dtype=np.float32) / RD))  # [32]
    pos = np.arange(S, dtype=np.float32)
    ang = inv_freq[:, None] * pos[None, :]              # [32, S]
    cos1, sin1 = np.cos(ang), np.sin(ang)
    cosA = np.tile(cos1, (4, 1))                        # [128, S]
    sinB = np.concatenate([-sin1, sin1, -sin1, sin1], axis=0)
    return cosA.astype(_bf), sinB.astype(_bf)


def _mask_tiles():
    """masks[d][k, q] = 1.0 if q >= d*128 + k else 0 (bf16, [4,128,512])."""
    k = np.arange(P)[:, None]
    q = np.arange(512)[None, :]
    m = np.stack([(q >= d * 128 + k) for d in range(4)]).astype(np.float32)
    return np.ascontiguousarray(m.transpose(1, 0, 2)).astype(_bf)


def _prep_core_inputs(c, x, W_kv_d, W_q_d, W_k_u, W_q_u, W_v_u, W_rope_k, W_rope_q,
                      W_o, cosA, sinB, masks):
    b = c // 4
    hg = c % 4
    heads = [4 * hg + j for j in range(HEADS_PER_CORE)]

    def tile_pmaj(w):
        # [ko*128, m] -> [128, ko, m] partition-major for contiguous DMA
        ko = w.shape[0] // P
        return np.ascontiguousarray(
            w.reshape(ko, P, w.shape[1]).transpose(1, 0, 2))

    xT = np.ascontiguousarray(x[b].T).astype(_bf)                  # [H, S]
    # xTq[p, nq*16+k, c] = xT[k*128+p, nq*512+c]
    xTq = np.ascontiguousarray(
        xT.reshape(16, P, 4, 512).transpose(1, 2, 0, 3).reshape(P, 64, 512))
    w_kvd = tile_pmaj(np.ascontiguousarray(W_kv_d.T).astype(_bf))
    w_qd = tile_pmaj(np.ascontiguousarray(W_q_d.T).astype(_bf))

    # w_rk: per pair, rows [h1 rope dims | h0 rope dims], then transpose
    blocks = []
    for p in range(2):
        g0, g1 = heads[2 * p], heads[2 * p + 1]
        blocks.append(W_rope_k[g1 * RD:(g1 + 1) * RD, :])
        blocks.append(W_rope_k[g0 * RD:(g0 + 1) * RD, :])
    w_rk = tile_pmaj(np.ascontiguousarray(np.concatenate(blocks, axis=0).T).astype(_bf))

    # w_qc: per local head 128 cols: even -> [content|rope], odd -> [rope|content]
    cols = []
    for j, g in enumerate(heads):
        c_blk = W_q_u[g * RD:(g + 1) * RD, :].T       # [LAT, 64]
        r_blk = W_rope_q[g * RD:(g + 1) * RD, :].T    # [LAT, 64]
        cols.extend([c_blk, r_blk] if j % 2 == 0 else [r_blk, c_blk])
    w_qc = tile_pmaj(np.ascontiguousarray(np.concatenate(cols, axis=1)).astype(_bf))

    # w_kc: per pair 128 cols: [h0 content | h1 content]
    cols = []
    for p in range(2):
        g0, g1 = heads[2 * p], heads[2 * p + 1]
        cols.append(W_k_u[g0 * RD:(g0 + 1) * RD, :].T)
        cols.append(W_k_u[g1 * RD:(g1 + 1) * RD, :].T)
    w_kc = tile_pmaj(np.ascontiguousarray(np.concatenate(cols, axis=1)).astype(_bf))

    # w_v: per pair 256 cols: [h0 v dims | h1 v dims]
    cols = []
    for p in range(2):
        g0, g1 = heads[2 * p], heads[2 * p + 1]
        cols.append(W_v_u[g0 * HD:(g0 + 1) * HD, :].T)
        cols.append(W_v_u[g1 * HD:(g1 + 1) * HD, :].T)
    w_v = tile_pmaj(np.ascontiguousarray(np.concatenate(cols, axis=1)).astype(_bf))

    d0 = heads[0] * HD
    w_o = tile_pmaj(np.ascontiguousarray(W_o[:, d0:d0 + 512].T).astype(_bf))

    return {
        "xTq": xTq, "w_kvd": w_kvd, "w_qd": w_qd, "w_rk": w_rk, "w_qc": w_qc,
        "w_kc": w_kc, "w_v": w_v, "w_o": w_o, "cosA": cosA, "sinB": sinB,
        "masks": masks,
    }


def make_in_maps(inputs):
    x = np.asarray(inputs["hidden_states"], dtype=np.float32)
    ws = {k: np.asarray(inputs[k], dtype=np.float32)
          for k in ("W_kv_d", "W_q_d", "W_k_u", "W_q_u", "W_v_u", "W_rope_k",
                    "W_rope_q", "W_o")}
    cosA, sinB = _rope_tables()
    masks = _mask_tiles()
    return [
        _prep_core_inputs(c, x, ws["W_kv_d"], ws["W_q_d"], ws["W_k_u"],
                          ws["W_q_u"], ws["W_v_u"], ws["W_rope_k"],
                          ws["W_rope_q"], ws["W_o"], cosA, sinB, masks)
        for c in range(N_CORES)
    ]


def assemble(results):
    """results: list of 8 dicts with 'out' [32, 128, 1024] bf16 partials.
    Tile t = qb2*16 + oc holds rows oc*128:(oc+1)*128, cols
    qb2*1024:(qb2+1)*1024 of the [H, S] transposed partial."""
    full = np.empty((B, S, H), dtype=np.float32)
    for b in range(B):
        acc = results[4 * b]["out"].astype(np.float32)
        for g in range(1, 4):
            acc = acc + results[4 * b + g]["out"]
        # [2, 16, 128, 1024] -> [H, S]
        ht = acc.reshape(2, 16, P, 1024).transpose(1, 2, 0, 3).reshape(H, S)
        full[b] = ht.T
    return full


def kernel(**inputs):
    nc = _get_nc()
    in_maps = make_in_maps(inputs)
    res = run_bass_kernel_spmd(nc, in_maps, core_ids=list(range(N_CORES)))
    return assemble(res.results)

